# revision 1
# baseline (speedup 1.0000x reference)
"""DiffPool (nn_DiffPool_4715874091424) Trainium2 Bass kernel, v2.

Math (reference jax, B=32, C=CR=32, N=L=160, GDEP=2, ALPHA=0.05):
  A  = rownorm(a + I), A' = rownorm(a.T + I)
  mixprop folding:  [embed; pool] = Wcat^T @ [x; M1 x; M2 x; ones] (bias row)
  with M1 = A + A', M2 = A^2 + A'^2 folded on host.
  s = softmax_v(pool);  x_new[c] = s[c]^T @ embed[c];
  a_new[c] = (s[c] @ a) @ s[c].

Device pipeline per batch element b (8 cores data-parallel over B, 4 b/core):
  y12:  y = [M1|M2]^T.T @ x_nm  (node matmuls, node-major in/out)
  perm: y_nm -> hcat rows 32:96 chan-major via SBUF->SBUF DMA
  mix:  per (l, v-tile): stationary = hcat[:, v-cols @ fixed l] (strided),
        moving = Wm [97, 64] -> psum [v, 64] -> node-major eg/xp directly
        (embed rows cast-evicted; pool rows exp-evicted).  No DRAM scratch.
  P2:   x_new raw = expP^T @ [eg|1] (161st col = softmax denom D);
        Dinv row-scale at evict; slg = s^T via PE transposes, Dinv-scaled;
        sng = expP * Dinv-flat (broadcast mult, replaces transpose-back);
        ttg = a^T s^T;  a_new^T = sng^T @ ttg (no post-scale).
  Outputs written f16 in [l, c, l'] layouts (contiguous DMA), host transposes
  to [c, l, l'] and casts f32.
"""

import sys

import numpy as np

if "/opt/trn_rl_repo" not in sys.path:
    sys.path.insert(0, "/opt/trn_rl_repo")

import concourse.bass as bass
import concourse.bacc as bacc
import concourse.mybir as mybir
import concourse.tile as tile
from concourse.bass_utils import run_bass_kernel_spmd
from concourse.masks import make_identity

F32 = mybir.dt.float32
F16 = mybir.dt.float16
AF = mybir.ActivationFunctionType
MUL = mybir.AluOpType.mult

B, C, N, L = 32, 32, 160, 160
NCORES = 8
BPC = B // NCORES
ALPHA, BETA = 0.05, 0.95
CL = C * L  # 5120
C1 = C * (L + 1)  # 5152
VT = [(0, 128), (128, 32)]
MTILES = [(0, 128), (128, 128), (256, 64)]
G3 = [(c0, min(3, C - c0)) for c0 in range(0, C, 3)]  # channel groups of 3


def build_nc():
    nc = bacc.Bacc("TRN2", target_bir_lowering=False, debug=False, num_devices=NCORES)
    xnm = nc.declare_dram_parameter("xnm", [BPC, N, C, L], F16, isOutput=False)
    xcm = nc.declare_dram_parameter("xcm", [BPC, C, N, L], F16, isOutput=False)
    mt = nc.declare_dram_parameter("mt", [N, 2 * N], F16, isOutput=False)
    wm = nc.declare_dram_parameter("wm", [3 * C + 1, 2 * C], F16, isOutput=False)
    am = nc.declare_dram_parameter("am", [N, N], F16, isOutput=False)
    xn_out = nc.declare_dram_parameter("xn", [BPC, L, C, L], F16, isOutput=True)
    an_out = nc.declare_dram_parameter("an", [BPC, L, C, N], F16, isOutput=True)
    # chan-major y scratch (node->chan layout transpose via DRAM roundtrip;
    # SBUF-side DMA APs require the partition dim outermost, so the permute
    # must go through DRAM)
    ys = nc.dram_tensor("ys", [BPC, C, 2 * N, L], F16)

    with tile.TileContext(nc) as tc:
        with (
            tc.tile_pool(name="consts", bufs=1) as pc,
            tc.tile_pool(name="work", bufs=1) as pw,
            tc.tile_pool(name="psum", bufs=1, space="PSUM") as pp,
        ):
            # ---- constants (loaded straight from DRAM slices) ----
            mtt = []
            for kt, (k0, ksz) in enumerate(((0, 128), (128, 32))):
                row = []
                for m0, msz in MTILES:
                    t = pc.tile([ksz, msz], F16, name=f"mtt{kt}_{m0}")
                    nc.sync.dma_start(t[:], mt[k0 : k0 + ksz, m0 : m0 + msz])
                    row.append(t)
                mtt.append(row)
            amt = []
            for kt, (k0, ksz) in enumerate(VT):
                row = []
                for m0, msz in VT:
                    t = pc.tile([ksz, msz], F16, name=f"amt{kt}_{m0}")
                    nc.sync.dma_start(t[:], am[k0 : k0 + ksz, m0 : m0 + msz])
                    row.append(t)
                amt.append(row)
            wmc = pc.tile([3 * C + 1, 2 * C], F16)
            nc.sync.dma_start(wmc[:], wm[:])
            ident = pc.tile([128, 128], F16)
            make_identity(nc, ident[:])

            # ---- persistent work tiles (reused across b; bufs=1) ----
            st = {}
            st["xnm0"] = pw.tile([128, CL], F16, tag="xnm0", name="xnm0")
            st["xnm1"] = pw.tile([32, CL], F16, tag="xnm1", name="xnm1")
            st["hcat"] = pw.tile([3 * C + 1, N * L], F16, tag="hcat", name="hcat")
            st["y0"] = pw.tile([128, CL], F16, tag="y0", name="y0")
            st["y1"] = pw.tile([128, CL], F16, tag="y1", name="y1")
            st["y2"] = pw.tile([64, CL], F16, tag="y2", name="y2")
            st["eg0"] = pw.tile([128, C1], F16, tag="eg0", name="eg0")
            st["eg1"] = pw.tile([32, C1], F16, tag="eg1", name="eg1")
            st["xp0"] = pw.tile([128, CL], F16, tag="xp0", name="xp0")
            st["xp1"] = pw.tile([32, CL], F16, tag="xp1", name="xp1")
            st["sl0"] = pw.tile([128, CL], F16, tag="sl0", name="sl0")
            st["sl1"] = pw.tile([32, CL], F16, tag="sl1", name="sl1")
            st["tt0"] = pw.tile([128, CL], F16, tag="tt0", name="tt0")
            st["tt1"] = pw.tile([32, CL], F16, tag="tt1", name="tt1")
            st["ob0"] = pw.tile([128, CL], F16, tag="ob0", name="ob0")
            st["ob1"] = pw.tile([32, CL], F16, tag="ob1", name="ob1")
            st["dv0"] = pw.tile([128, C], F32, tag="dv0", name="dv0")
            st["dv1"] = pw.tile([32, C], F32, tag="dv1", name="dv1")
            st["dh0"] = pw.tile([128, C], F16, tag="dh0", name="dh0")
            st["dh1"] = pw.tile([32, C], F16, tag="dh1", name="dh1")

            # ones column of eg (col 160 of each channel block); constant
            nc.vector.memset(
                st["eg0"][:].rearrange("p (c q) -> p c q", q=L + 1)[:, :, L : L + 1],
                1.0,
            )
            nc.vector.memset(
                st["eg1"][:].rearrange("p (c q) -> p c q", q=L + 1)[:, :, L : L + 1],
                1.0,
            )
            # ones row of hcat (bias row 96); constant across b
            nc.gpsimd.memset(st["hcat"][3 * C : 3 * C + 1, :], 1.0)

            # PE warm-up: back-to-back matmuls release the HAM clock gate while
            # the first x tiles stream in
            warm = pc.tile([128, 512], F16, name="warm")
            nc.vector.memset(warm[:], 0.125)

            _xin_nm(nc, st, xnm, 0)
            _xin_cm(nc, st, xcm, 0)
            for _ in range(24):
                wps = pp.tile([128, 512], F32, tag="psA", name="psA", bufs=3)
                nc.tensor.matmul(wps[:], warm[:, 0:128], warm[:], start=True, stop=True)

            for b in range(BPC):
                _y12(nc, pp, st, mtt, ys, b)
                if b + 1 < BPC:
                    _xin_nm(nc, st, xnm, b + 1)
                if b > 0:
                    _p2(nc, pp, st, amt, ident, xn_out, an_out, b - 1)
                _mix(nc, pp, st, wmc, b)
                if b + 1 < BPC:
                    _xin_cm(nc, st, xcm, b + 1)
            _p2(nc, pp, st, amt, ident, xn_out, an_out, BPC - 1)

    return nc


def _psA(pp, shape, dt=F32):
    return pp.tile(shape, dt, tag="psA", name="psA", bufs=3)


def _psB(pp, shape, dt=F32):
    return pp.tile(shape, dt, tag="psB", name="psB", bufs=3)


def _psT(pp, shape, dt=F16):
    return pp.tile(shape, dt, tag="psT", name="psT", bufs=2)


def _xin_nm(nc, st, xnm, b):
    nc.sync.dma_start(
        st["xnm0"][:].rearrange("p (c l) -> p c l", c=C), xnm[b][0:128]
    )
    nc.sync.dma_start(
        st["xnm1"][:].rearrange("p (c l) -> p c l", c=C), xnm[b][128:160]
    )


def _xin_cm(nc, st, xcm, b):
    # x chan-major -> hcat rows 0:32 (contiguous)
    nc.sync.dma_start(
        st["hcat"][0:C, :].rearrange("c (v l) -> c v l", v=N), xcm[b]
    )


def _y12(nc, pp, st, mtt, ys, b):
    """y = [M1|M2]^T.T @ x_nm -> y_nm tiles; SBUF->SBUF permute into hcat."""
    Y = [st["y0"], st["y1"], st["y2"]]
    xs = [st["xnm0"], st["xnm1"]]
    for mi, (m0, msz) in enumerate(MTILES):
        for sg in range(10):
            ps = _psA(pp, [128, 512])
            for kt in range(2):
                nc.tensor.matmul(
                    ps[:msz, :],
                    mtt[kt][mi][:],
                    xs[kt][:, sg * 512 : (sg + 1) * 512],
                    start=(kt == 0),
                    stop=(kt == 1),
                )
            nc.scalar.copy(Y[mi][:msz, sg * 512 : (sg + 1) * 512], ps[:msz, :])

    # permute y (node-major) -> chan-major via DRAM roundtrip
    hc = st["hcat"]
    nc.gpsimd.dma_start(
        ys[b][:, 0:128, :].rearrange("c v l -> v c l"),
        st["y0"][:].rearrange("v (c l) -> v c l", c=C),
    )
    nc.gpsimd.dma_start(
        ys[b][:, 128:160, :].rearrange("c v l -> v c l"),
        st["y1"][0:32, :].rearrange("v (c l) -> v c l", c=C),
    )
    nc.gpsimd.dma_start(
        ys[b][:, 160:256, :].rearrange("c v l -> v c l"),
        st["y1"][32:128, :].rearrange("v (c l) -> v c l", c=C),
    )
    nc.gpsimd.dma_start(
        ys[b][:, 256:320, :].rearrange("c v l -> v c l"),
        st["y2"][:].rearrange("v (c l) -> v c l", c=C),
    )
    # read back chan-major into hcat rows 32:96 (contiguous both sides)
    nc.sync.dma_start(
        hc[C : 2 * C, :].rearrange("c (v l) -> c v l", v=N),
        ys[b][:, 0:N, :],
    )
    nc.sync.dma_start(
        hc[2 * C : 3 * C, :].rearrange("c (v l) -> c v l", v=N),
        ys[b][:, N : 2 * N, :],
    )


def _mix(nc, pp, st, wmc, b):
    """Channel mix, node-major output: per (l, v-tile) stationary=hcat cols."""
    hc = st["hcat"]
    hv = hc[:].rearrange("p (v l) -> p l v", l=L)  # [97, l, v] strided view
    egs = [st["eg0"], st["eg1"]]
    xps = [st["xp0"], st["xp1"]]
    for vti, (v0, vsz) in enumerate(VT):
        for lg in range(20):  # l-groups of 8 -> one psum bank
            ps = _psA(pp, [128, 512])
            for li in range(8):
                l = lg * 8 + li
                nc.tensor.matmul(
                    ps[:vsz, li * 64 : (li + 1) * 64],
                    hv[:, l, v0 : v0 + vsz],
                    wmc[:],
                    start=True,
                    stop=True,
                )
            pv = ps[:vsz, :].rearrange("p (l c) -> p c l", c=64)
            # embed rows -> eg (cast, DVE), pool rows -> xp (exp, ACT)
            nc.vector.tensor_copy(
                egs[vti][:]
                .rearrange("p (c q) -> p c q", q=L + 1)[:, :, lg * 8 : lg * 8 + 8],
                pv[:, 0:C, :],
            )
            nc.scalar.activation(
                xps[vti][:]
                .rearrange("p (c q) -> p c q", q=L)[:, :, lg * 8 : lg * 8 + 8],
                pv[:, C : 2 * C, :],
                AF.Exp,
            )


def _p2(nc, pp, st, amt, ident, xn_out, an_out, b):
    """x_new + softmax scales + ttg + a_new^T + output DMAs for batch elem b."""
    egs = [st["eg0"], st["eg1"]]
    xps = [st["xp0"], st["xp1"]]
    slg = [st["sl0"], st["sl1"]]
    ttg = [st["tt0"], st["tt1"]]
    osb = [st["ob0"], st["ob1"]]
    dvs = [st["dv0"], st["dv1"]]
    dhs = [st["dh0"], st["dh1"]]

    # ---- x_new raw + D + Dinv + scaled evict ----
    for c0, gc in G3:
        for mi, (m0, msz) in enumerate(VT):
            ps = _psB(pp, [128, 512])
            for ci in range(gc):
                c = c0 + ci
                for kt, (k0, ksz) in enumerate(VT):
                    nc.tensor.matmul(
                        ps[:msz, ci * 161 : ci * 161 + 161],
                        xps[kt][:, c * L + m0 : c * L + m0 + msz],
                        egs[kt][:, c * (L + 1) : (c + 1) * (L + 1)],
                        start=(kt == 0),
                        stop=(kt == 1),
                    )
            pq = ps[:msz, : gc * 161].rearrange("p (c q) -> p c q", q=161)
            nc.vector.reciprocal(
                dvs[mi][:msz, c0 : c0 + gc], pq[:, :, 160:161].rearrange("p c q -> p (c q)")
            )
            # out = raw * Dinv[l(part), c]  (broadcast along l')
            in1 = (
                dvs[mi][:msz, c0 : c0 + gc]
                .unsqueeze(2)
                .broadcast_to([msz, gc, L])
            )
            nc.vector.tensor_tensor(
                osb[mi][:msz, c0 * L : (c0 + gc) * L].rearrange(
                    "p (c q) -> p c q", q=L
                ),
                pq[:, :, 0:L],
                in1,
                MUL,
            )
    # f16 copy of Dinv for the cheap f16 multiplies
    for mi, (m0, msz) in enumerate(VT):
        nc.vector.tensor_copy(dhs[mi][:msz, :], dvs[mi][:msz, :])

    # ---- T1: slg = transpose(expP) * Dinv (l-major s^T) ----
    for c0, gc in G3:
        for lt, (l0, lsz) in enumerate(VT):
            ps = _psT(pp, [128, 512], F16)
            for ci in range(gc):
                c = c0 + ci
                for kt, (k0, ksz) in enumerate(VT):
                    nc.tensor.transpose(
                        ps[:lsz, ci * L + k0 : ci * L + k0 + ksz],
                        xps[kt][:, c * L + l0 : c * L + l0 + lsz],
                        ident[:ksz, :ksz],
                    )
            in1 = (
                dhs[lt][:lsz, c0 : c0 + gc]
                .unsqueeze(2)
                .broadcast_to([lsz, gc, N])
            )
            nc.vector.tensor_tensor(
                slg[lt][:lsz, c0 * N : (c0 + gc) * N].rearrange(
                    "p (c q) -> p c q", q=N
                ),
                ps[:lsz, : gc * N].rearrange("p (c q) -> p c q", q=N),
                in1,
                MUL,
            )

    # ---- ttg = a^T s^T ----
    for mi, (m0, msz) in enumerate(VT):
        for ch in range(10):
            ps = _psB(pp, [128, 512])
            for kt in range(2):
                nc.tensor.matmul(
                    ps[:msz, :],
                    amt[kt][mi][:],
                    slg[kt][:, ch * 512 : (ch + 1) * 512],
                    start=(kt == 0),
                    stop=(kt == 1),
                )
            nc.scalar.copy(ttg[mi][:msz, ch * 512 : (ch + 1) * 512], ps[:msz, :])

    # ---- xn out (osb holds x_new now) ----
    nc.sync.dma_start(
        xn_out[b][0:128], osb[0][:].rearrange("p (c l) -> p c l", l=L)
    )
    nc.sync.dma_start(
        xn_out[b][128:160], osb[1][:].rearrange("p (c l) -> p c l", l=L)
    )

    # ---- a_new^T raw = expP^T @ ttg;  Dinv[l'] row-scale at evict ----
    # (a_new[v,l'] = Dinv[l'] * sum_j expP[j,l'] tT[j,v]: the s-column scale
    #  commutes out of the j-sum as a per-output-partition factor)
    for c0, gc in G3:
        for lt, (l0, lsz) in enumerate(VT):
            ps = _psB(pp, [128, 512])
            for ci in range(gc):
                c = c0 + ci
                for jt, (j0, jsz) in enumerate(VT):
                    nc.tensor.matmul(
                        ps[:lsz, ci * N : (ci + 1) * N],
                        xps[jt][:, c * L + l0 : c * L + l0 + lsz],
                        ttg[jt][:, c * N : (c + 1) * N],
                        start=(jt == 0),
                        stop=(jt == 1),
                    )
            in1 = (
                dvs[lt][:lsz, c0 : c0 + gc]
                .unsqueeze(2)
                .broadcast_to([lsz, gc, N])
            )
            nc.vector.tensor_tensor(
                osb[lt][:lsz, c0 * N : (c0 + gc) * N].rearrange(
                    "p (c q) -> p c q", q=N
                ),
                ps[:lsz, : gc * N].rearrange("p (c q) -> p c q", q=N),
                in1,
                MUL,
            )

    # ---- an out ----
    nc.sync.dma_start(
        an_out[b][0:128], osb[0][:].rearrange("p (c l) -> p c l", l=N)
    )
    nc.sync.dma_start(
        an_out[b][128:160], osb[1][:].rearrange("p (c l) -> p c l", l=N)
    )


def _host_prep(x, a, We, be, Wp, bp):
    a = np.asarray(a, np.float64)
    I = np.eye(N, dtype=np.float64)
    A1 = (a + I) / (a + I).sum(1, keepdims=True)
    A2 = (a.T + I) / (a.T + I).sum(1, keepdims=True)
    M1 = A1 + A2
    M2 = A1 @ A1 + A2 @ A2
    MT = np.concatenate([M1.T, M2.T], axis=1).astype(np.float16)  # [N, 2N]

    def fold(W):
        W = np.asarray(W, np.float64)
        W0, W1, W2 = W[:, :C], W[:, C : 2 * C], W[:, 2 * C :]
        F0 = 2.0 * (W0 + ALPHA * W1 + ALPHA * W2)
        F1 = BETA * W1 + ALPHA * BETA * W2
        F2 = BETA * BETA * W2
        return F0, F1, F2

    E0, E1, E2 = fold(We)
    P0, P1, P2 = fold(Wp)
    Wcat = np.block([[E0.T, P0.T], [E1.T, P1.T], [E2.T, P2.T]])  # [96, 64]
    brow = np.concatenate([2.0 * np.asarray(be), 2.0 * np.asarray(bp)])[None, :]
    Wm = np.concatenate([Wcat, brow], axis=0).astype(np.float16)  # [97, 64]
    return MT, Wm, np.asarray(a, np.float16)


def _install_ntff_shim():
    """Provide antenv.axon_hooks (missing in this image) so
    run_bass_kernel_spmd(trace=True) can drive NTFF profiling via the
    axon PJRT .so. No-op if anything is unavailable."""
    import contextlib
    import ctypes
    import types

    try:
        import antenv  # noqa: F401

        try:
            from antenv.axon_hooks import get_axon_ntff_profile_hook  # noqa: F401

            return
        except ImportError:
            pass
        lib = ctypes.CDLL("/opt/axon/libaxon_pjrt.so")
        if not hasattr(lib, "axon_start_nrt_profile"):
            return
        lib.axon_start_nrt_profile.argtypes = [
            ctypes.POINTER(ctypes.c_int64),
            ctypes.c_size_t,
        ]
        lib.axon_start_nrt_profile.restype = ctypes.c_int64
        lib.axon_stop_nrt_profile.argtypes = [ctypes.c_char_p]
        lib.axon_stop_nrt_profile.restype = ctypes.c_int64

        @contextlib.contextmanager
        def _hook(output_dir, device_ids):
            import jax

            jax.devices()
            if device_ids:
                ids = (ctypes.c_int64 * len(device_ids))(*device_ids)
                rc = lib.axon_start_nrt_profile(ids, len(device_ids))
            else:
                rc = lib.axon_start_nrt_profile(None, 0)
            if rc != 0:
                raise RuntimeError(f"axon_start_nrt_profile rc={rc}")
            try:
                yield
            finally:
                n = lib.axon_stop_nrt_profile(str(output_dir).encode())
                print(f"ntff profile: {n} file(s) -> {output_dir}", file=sys.stderr)

        holder = {"h": _hook}
        mod = types.ModuleType("antenv.axon_hooks")
        mod.get_axon_ntff_profile_hook = lambda: holder["h"]
        mod.set_axon_ntff_profile_hook = lambda h: holder.__setitem__("h", h)
        sys.modules["antenv.axon_hooks"] = mod
        antenv.axon_hooks = mod
    except Exception as e:  # pragma: no cover
        print(f"ntff shim unavailable: {e}", file=sys.stderr)


_NC_CACHE = {}


def _get_nc():
    if "nc" not in _NC_CACHE:
        nc = build_nc()
        nc.compile()
        _NC_CACHE["nc"] = nc
    return _NC_CACHE["nc"]


def run_spmd(x, a, We, be, Wp, bp, trace=False):
    if trace:
        _install_ntff_shim()
    x16 = np.ascontiguousarray(np.asarray(x, np.float16))
    xnm = np.ascontiguousarray(np.transpose(x16, (0, 2, 1, 3)).reshape(
        NCORES, BPC, N, C, L
    ))
    xcm = x16.reshape(NCORES, BPC, C, N, L)
    MT, Wm, a16 = _host_prep(x, a, We, be, Wp, bp)
    nc = _get_nc()
    in_maps = [
        {
            "xnm": xnm[i],
            "xcm": xcm[i],
            "mt": MT,
            "wm": Wm,
            "am": a16,
        }
        for i in range(NCORES)
    ]
    res = run_bass_kernel_spmd(nc, in_maps, list(range(NCORES)), trace=trace)
    xn_s = np.concatenate([res.results[i]["xn"] for i in range(NCORES)], axis=0)
    an_s = np.concatenate([res.results[i]["an"] for i in range(NCORES)], axis=0)
    # [B, L, C, L'] -> [B, C, L, L'] ; [B, L', C, N] -> [B, C, N, L']
    xn = np.ascontiguousarray(np.transpose(xn_s, (0, 2, 1, 3))).astype(np.float32)
    an = np.ascontiguousarray(np.transpose(an_s, (0, 2, 3, 1))).astype(np.float32)
    return (xn, an), res


def kernel(x, a, We, be, Wp, bp):
    (xn, an), _ = run_spmd(x, a, We, be, Wp, bp, trace=False)
    return (xn, an)



# revision 3
# speedup vs baseline: 1.0063x; 1.0063x over previous
"""DiffPool (nn_DiffPool_4715874091424) Trainium2 Bass kernel, v2.

Math (reference jax, B=32, C=CR=32, N=L=160, GDEP=2, ALPHA=0.05):
  A  = rownorm(a + I), A' = rownorm(a.T + I)
  mixprop folding:  [embed; pool] = Wcat^T @ [x; M1 x; M2 x; ones] (bias row)
  with M1 = A + A', M2 = A^2 + A'^2 folded on host.
  s = softmax_v(pool);  x_new[c] = s[c]^T @ embed[c];
  a_new[c] = (s[c] @ a) @ s[c].

Device pipeline per batch element b (8 cores data-parallel over B, 4 b/core):
  y12:  y = [M1|M2]^T.T @ x_nm  (node matmuls, node-major in/out)
  perm: y_nm -> hcat rows 32:96 chan-major via SBUF->SBUF DMA
  mix:  per (l, v-tile): stationary = hcat[:, v-cols @ fixed l] (strided),
        moving = Wm [97, 64] -> psum [v, 64] -> node-major eg/xp directly
        (embed rows cast-evicted; pool rows exp-evicted).  No DRAM scratch.
  P2:   x_new raw = expP^T @ [eg|1] (161st col = softmax denom D);
        Dinv row-scale at evict; slg = s^T via PE transposes, Dinv-scaled;
        sng = expP * Dinv-flat (broadcast mult, replaces transpose-back);
        ttg = a^T s^T;  a_new^T = sng^T @ ttg (no post-scale).
  Outputs written f16 in [l, c, l'] layouts (contiguous DMA), host transposes
  to [c, l, l'] and casts f32.
"""

import sys

import numpy as np

if "/opt/trn_rl_repo" not in sys.path:
    sys.path.insert(0, "/opt/trn_rl_repo")

import concourse.bass as bass
import concourse.bacc as bacc
import concourse.mybir as mybir
import concourse.tile as tile
from concourse.bass_utils import run_bass_kernel_spmd
from concourse.masks import make_identity

F32 = mybir.dt.float32
F16 = mybir.dt.float16
AF = mybir.ActivationFunctionType
MUL = mybir.AluOpType.mult

B, C, N, L = 32, 32, 160, 160
NCORES = 8
BPC = B // NCORES
ALPHA, BETA = 0.05, 0.95
CL = C * L  # 5120
C1 = C * (L + 1)  # 5152
VT = [(0, 128), (128, 32)]
MTILES = [(0, 128), (128, 128), (256, 64)]
G3 = [(c0, min(3, C - c0)) for c0 in range(0, C, 3)]  # channel groups of 3


def build_nc():
    nc = bacc.Bacc("TRN2", target_bir_lowering=False, debug=False, num_devices=NCORES)
    xnm = nc.declare_dram_parameter("xnm", [BPC, N, C, L], F16, isOutput=False)
    xcm = nc.declare_dram_parameter("xcm", [BPC, C, N, L], F16, isOutput=False)
    mt = nc.declare_dram_parameter("mt", [N, 2 * N], F16, isOutput=False)
    wm = nc.declare_dram_parameter("wm", [3 * C + 1, 2 * C], F16, isOutput=False)
    am = nc.declare_dram_parameter("am", [N, N], F16, isOutput=False)
    xn_out = nc.declare_dram_parameter("xn", [BPC, L, C, L], F16, isOutput=True)
    an_out = nc.declare_dram_parameter("an", [BPC, L, C, N], F16, isOutput=True)
    # chan-major y scratch (node->chan layout transpose via DRAM roundtrip;
    # SBUF-side DMA APs require the partition dim outermost, so the permute
    # must go through DRAM)
    ys = nc.dram_tensor("ys", [BPC, C, 2 * N, L], F16)

    with tile.TileContext(nc) as tc:
        with (
            tc.tile_pool(name="consts", bufs=1) as pc,
            tc.tile_pool(name="work", bufs=1) as pw,
            tc.tile_pool(name="psum", bufs=1, space="PSUM") as pp,
        ):
            # ---- constants (loaded straight from DRAM slices) ----
            mtt = []
            for kt, (k0, ksz) in enumerate(((0, 128), (128, 32))):
                row = []
                for m0, msz in MTILES:
                    t = pc.tile([ksz, msz], F16, name=f"mtt{kt}_{m0}")
                    nc.scalar.dma_start(t[:], mt[k0 : k0 + ksz, m0 : m0 + msz])
                    row.append(t)
                mtt.append(row)
            amt = []
            for kt, (k0, ksz) in enumerate(VT):
                row = []
                for m0, msz in VT:
                    t = pc.tile([ksz, msz], F16, name=f"amt{kt}_{m0}")
                    nc.scalar.dma_start(t[:], am[k0 : k0 + ksz, m0 : m0 + msz])
                    row.append(t)
                amt.append(row)
            wmc = pc.tile([3 * C + 1, 2 * C], F16)
            nc.scalar.dma_start(wmc[:], wm[:])
            ident = pc.tile([128, 128], F16)
            make_identity(nc, ident[:])

            # ---- persistent work tiles (reused across b; bufs=1) ----
            st = {}
            st["xnm0"] = pw.tile([128, CL], F16, tag="xnm0", name="xnm0")
            st["xnm1"] = pw.tile([32, CL], F16, tag="xnm1", name="xnm1")
            st["hcat"] = pw.tile([3 * C + 1, N * L], F16, tag="hcat", name="hcat")
            st["y0"] = pw.tile([128, CL], F16, tag="y0", name="y0")
            st["y1"] = pw.tile([128, CL], F16, tag="y1", name="y1")
            st["y2"] = pw.tile([64, CL], F16, tag="y2", name="y2")
            st["eg0"] = pw.tile([128, C1], F16, tag="eg0", name="eg0")
            st["eg1"] = pw.tile([32, C1], F16, tag="eg1", name="eg1")
            st["xp0"] = pw.tile([128, CL], F16, tag="xp0", name="xp0")
            st["xp1"] = pw.tile([32, CL], F16, tag="xp1", name="xp1")
            st["sl0"] = pw.tile([128, CL], F16, tag="sl0", name="sl0")
            st["sl1"] = pw.tile([32, CL], F16, tag="sl1", name="sl1")
            st["tt0"] = pw.tile([128, CL], F16, tag="tt0", name="tt0")
            st["tt1"] = pw.tile([32, CL], F16, tag="tt1", name="tt1")
            st["ob0"] = pw.tile([128, CL], F16, tag="ob0", name="ob0")
            st["ob1"] = pw.tile([32, CL], F16, tag="ob1", name="ob1")
            st["dv0"] = pw.tile([128, C], F32, tag="dv0", name="dv0")
            st["dv1"] = pw.tile([32, C], F32, tag="dv1", name="dv1")
            st["dh0"] = pw.tile([128, C], F16, tag="dh0", name="dh0")
            st["dh1"] = pw.tile([32, C], F16, tag="dh1", name="dh1")

            # ones column of eg (col 160 of each channel block); constant
            nc.vector.memset(
                st["eg0"][:].rearrange("p (c q) -> p c q", q=L + 1)[:, :, L : L + 1],
                1.0,
            )
            nc.vector.memset(
                st["eg1"][:].rearrange("p (c q) -> p c q", q=L + 1)[:, :, L : L + 1],
                1.0,
            )
            # ones row of hcat (bias row 96); constant across b
            nc.gpsimd.memset(st["hcat"][3 * C : 3 * C + 1, :], 1.0)

            # PE warm-up: back-to-back matmuls release the HAM clock gate while
            # the first x tiles stream in
            warm = pc.tile([128, 512], F16, name="warm")
            nc.vector.memset(warm[:], 0.125)

            _xin_nm(nc, st, xnm, 0)
            _xin_cm(nc, st, xcm, 0)
            for _ in range(52):
                wps = pp.tile([128, 512], F32, tag="psA", name="psA", bufs=3)
                nc.tensor.matmul(wps[:], warm[:, 0:128], warm[:], start=True, stop=True)

            for b in range(BPC):
                _y12(nc, pp, st, mtt, ys, b)
                if b + 1 < BPC:
                    _xin_nm(nc, st, xnm, b + 1)
                if b == 0:
                    # bridge the y12(0) -> mix(0) DRAM-permute wait with filler
                    # matmuls so the HAM clock gate stays released
                    for _ in range(96):
                        wps = pp.tile([128, 512], F32, tag="psA", name="psA", bufs=3)
                        nc.tensor.matmul(
                            wps[:], warm[:, 0:128], warm[:], start=True, stop=True
                        )
                if b > 0:
                    _p2(nc, pp, st, amt, ident, xn_out, an_out, b - 1)
                _mix(nc, pp, st, wmc, b)
                if b + 1 < BPC:
                    _xin_cm(nc, st, xcm, b + 1)
            _p2(nc, pp, st, amt, ident, xn_out, an_out, BPC - 1)

    return nc


def _psA(pp, shape, dt=F32):
    return pp.tile(shape, dt, tag="psA", name="psA", bufs=3)


def _psB(pp, shape, dt=F32):
    return pp.tile(shape, dt, tag="psB", name="psB", bufs=3)


def _psT(pp, shape, dt=F16):
    return pp.tile(shape, dt, tag="psT", name="psT", bufs=2)


def _xin_nm(nc, st, xnm, b):
    nc.sync.dma_start(
        st["xnm0"][:].rearrange("p (c l) -> p c l", c=C), xnm[b][0:128]
    )
    nc.sync.dma_start(
        st["xnm1"][:].rearrange("p (c l) -> p c l", c=C), xnm[b][128:160]
    )


def _xin_cm(nc, st, xcm, b):
    # x chan-major -> hcat rows 0:32 (contiguous)
    nc.scalar.dma_start(
        st["hcat"][0:C, :].rearrange("c (v l) -> c v l", v=N), xcm[b]
    )


def _y12(nc, pp, st, mtt, ys, b):
    """y = [M1|M2]^T.T @ x_nm -> y_nm tiles; SBUF->SBUF permute into hcat."""
    Y = [st["y0"], st["y1"], st["y2"]]
    xs = [st["xnm0"], st["xnm1"]]
    for mi, (m0, msz) in enumerate(MTILES):
        for sg in range(10):
            ps = _psA(pp, [128, 512])
            for kt in range(2):
                nc.tensor.matmul(
                    ps[:msz, :],
                    mtt[kt][mi][:],
                    xs[kt][:, sg * 512 : (sg + 1) * 512],
                    start=(kt == 0),
                    stop=(kt == 1),
                )
            nc.scalar.copy(Y[mi][:msz, sg * 512 : (sg + 1) * 512], ps[:msz, :])

    # permute y (node-major) -> chan-major via DRAM roundtrip
    hc = st["hcat"]
    nc.gpsimd.dma_start(
        ys[b][:, 0:128, :].rearrange("c v l -> v c l"),
        st["y0"][:].rearrange("v (c l) -> v c l", c=C),
    )
    nc.gpsimd.dma_start(
        ys[b][:, 128:160, :].rearrange("c v l -> v c l"),
        st["y1"][0:32, :].rearrange("v (c l) -> v c l", c=C),
    )
    nc.gpsimd.dma_start(
        ys[b][:, 160:256, :].rearrange("c v l -> v c l"),
        st["y1"][32:128, :].rearrange("v (c l) -> v c l", c=C),
    )
    nc.gpsimd.dma_start(
        ys[b][:, 256:320, :].rearrange("c v l -> v c l"),
        st["y2"][:].rearrange("v (c l) -> v c l", c=C),
    )
    # read back chan-major into hcat rows 32:96 (contiguous both sides)
    nc.sync.dma_start(
        hc[C : 2 * C, :].rearrange("c (v l) -> c v l", v=N),
        ys[b][:, 0:N, :],
    )
    nc.sync.dma_start(
        hc[2 * C : 3 * C, :].rearrange("c (v l) -> c v l", v=N),
        ys[b][:, N : 2 * N, :],
    )


def _mix(nc, pp, st, wmc, b):
    """Channel mix, node-major output: per (l, v-tile) stationary=hcat cols."""
    hc = st["hcat"]
    hv = hc[:].rearrange("p (v l) -> p l v", l=L)  # [97, l, v] strided view
    egs = [st["eg0"], st["eg1"]]
    xps = [st["xp0"], st["xp1"]]
    for vti, (v0, vsz) in enumerate(VT):
        for lg in range(20):  # l-groups of 8 -> one psum bank
            ps = _psA(pp, [128, 512])
            for li in range(8):
                l = lg * 8 + li
                nc.tensor.matmul(
                    ps[:vsz, li * 64 : (li + 1) * 64],
                    hv[:, l, v0 : v0 + vsz],
                    wmc[:],
                    start=True,
                    stop=True,
                )
            pv = ps[:vsz, :].rearrange("p (l c) -> p c l", c=64)
            # embed rows -> eg (cast, DVE), pool rows -> xp (exp, ACT)
            nc.vector.tensor_copy(
                egs[vti][:]
                .rearrange("p (c q) -> p c q", q=L + 1)[:, :, lg * 8 : lg * 8 + 8],
                pv[:, 0:C, :],
            )
            nc.scalar.activation(
                xps[vti][:]
                .rearrange("p (c q) -> p c q", q=L)[:, :, lg * 8 : lg * 8 + 8],
                pv[:, C : 2 * C, :],
                AF.Exp,
            )


def _p2(nc, pp, st, amt, ident, xn_out, an_out, b):
    """x_new + softmax scales + ttg + a_new^T + output DMAs for batch elem b."""
    egs = [st["eg0"], st["eg1"]]
    xps = [st["xp0"], st["xp1"]]
    slg = [st["sl0"], st["sl1"]]
    ttg = [st["tt0"], st["tt1"]]
    osb = [st["ob0"], st["ob1"]]
    dvs = [st["dv0"], st["dv1"]]
    dhs = [st["dh0"], st["dh1"]]

    # ---- x_new raw + D + Dinv + scaled evict ----
    for c0, gc in G3:
        for mi, (m0, msz) in enumerate(VT):
            ps = _psB(pp, [128, 512])
            for ci in range(gc):
                c = c0 + ci
                for kt, (k0, ksz) in enumerate(VT):
                    nc.tensor.matmul(
                        ps[:msz, ci * 161 : ci * 161 + 161],
                        xps[kt][:, c * L + m0 : c * L + m0 + msz],
                        egs[kt][:, c * (L + 1) : (c + 1) * (L + 1)],
                        start=(kt == 0),
                        stop=(kt == 1),
                    )
            pq = ps[:msz, : gc * 161].rearrange("p (c q) -> p c q", q=161)
            nc.vector.reciprocal(
                dvs[mi][:msz, c0 : c0 + gc], pq[:, :, 160:161].rearrange("p c q -> p (c q)")
            )
            # out = raw * Dinv[l(part), c]  (broadcast along l')
            in1 = (
                dvs[mi][:msz, c0 : c0 + gc]
                .unsqueeze(2)
                .broadcast_to([msz, gc, L])
            )
            nc.vector.tensor_tensor(
                osb[mi][:msz, c0 * L : (c0 + gc) * L].rearrange(
                    "p (c q) -> p c q", q=L
                ),
                pq[:, :, 0:L],
                in1,
                MUL,
            )
    # f16 copy of Dinv for the cheap f16 multiplies
    for mi, (m0, msz) in enumerate(VT):
        nc.vector.tensor_copy(dhs[mi][:msz, :], dvs[mi][:msz, :])

    # ---- T1: slg = transpose(expP) * Dinv (l-major s^T) ----
    for c0, gc in G3:
        for lt, (l0, lsz) in enumerate(VT):
            ps = _psT(pp, [128, 512], F16)
            for ci in range(gc):
                c = c0 + ci
                for kt, (k0, ksz) in enumerate(VT):
                    nc.tensor.transpose(
                        ps[:lsz, ci * L + k0 : ci * L + k0 + ksz],
                        xps[kt][:, c * L + l0 : c * L + l0 + lsz],
                        ident[:ksz, :ksz],
                    )
            in1 = (
                dhs[lt][:lsz, c0 : c0 + gc]
                .unsqueeze(2)
                .broadcast_to([lsz, gc, N])
            )
            nc.vector.tensor_tensor(
                slg[lt][:lsz, c0 * N : (c0 + gc) * N].rearrange(
                    "p (c q) -> p c q", q=N
                ),
                ps[:lsz, : gc * N].rearrange("p (c q) -> p c q", q=N),
                in1,
                MUL,
            )

    # ---- ttg = a^T s^T ----
    for mi, (m0, msz) in enumerate(VT):
        for ch in range(10):
            ps = _psB(pp, [128, 512])
            for kt in range(2):
                nc.tensor.matmul(
                    ps[:msz, :],
                    amt[kt][mi][:],
                    slg[kt][:, ch * 512 : (ch + 1) * 512],
                    start=(kt == 0),
                    stop=(kt == 1),
                )
            nc.scalar.copy(ttg[mi][:msz, ch * 512 : (ch + 1) * 512], ps[:msz, :])

    # ---- xn out (osb holds x_new now) ----
    nc.sync.dma_start(
        xn_out[b][0:128], osb[0][:].rearrange("p (c l) -> p c l", l=L)
    )
    nc.sync.dma_start(
        xn_out[b][128:160], osb[1][:].rearrange("p (c l) -> p c l", l=L)
    )

    # ---- a_new^T raw = expP^T @ ttg;  Dinv[l'] row-scale at evict ----
    # (a_new[v,l'] = Dinv[l'] * sum_j expP[j,l'] tT[j,v]: the s-column scale
    #  commutes out of the j-sum as a per-output-partition factor)
    for c0, gc in G3:
        for lt, (l0, lsz) in enumerate(VT):
            ps = _psB(pp, [128, 512])
            for ci in range(gc):
                c = c0 + ci
                for jt, (j0, jsz) in enumerate(VT):
                    nc.tensor.matmul(
                        ps[:lsz, ci * N : (ci + 1) * N],
                        xps[jt][:, c * L + l0 : c * L + l0 + lsz],
                        ttg[jt][:, c * N : (c + 1) * N],
                        start=(jt == 0),
                        stop=(jt == 1),
                    )
            in1 = (
                dvs[lt][:lsz, c0 : c0 + gc]
                .unsqueeze(2)
                .broadcast_to([lsz, gc, N])
            )
            nc.vector.tensor_tensor(
                osb[lt][:lsz, c0 * N : (c0 + gc) * N].rearrange(
                    "p (c q) -> p c q", q=N
                ),
                ps[:lsz, : gc * N].rearrange("p (c q) -> p c q", q=N),
                in1,
                MUL,
            )

    # ---- an out ----
    nc.sync.dma_start(
        an_out[b][0:128], osb[0][:].rearrange("p (c l) -> p c l", l=N)
    )
    nc.sync.dma_start(
        an_out[b][128:160], osb[1][:].rearrange("p (c l) -> p c l", l=N)
    )


def _host_prep(x, a, We, be, Wp, bp):
    a = np.asarray(a, np.float64)
    I = np.eye(N, dtype=np.float64)
    A1 = (a + I) / (a + I).sum(1, keepdims=True)
    A2 = (a.T + I) / (a.T + I).sum(1, keepdims=True)
    M1 = A1 + A2
    M2 = A1 @ A1 + A2 @ A2
    MT = np.concatenate([M1.T, M2.T], axis=1).astype(np.float16)  # [N, 2N]

    def fold(W):
        W = np.asarray(W, np.float64)
        W0, W1, W2 = W[:, :C], W[:, C : 2 * C], W[:, 2 * C :]
        F0 = 2.0 * (W0 + ALPHA * W1 + ALPHA * W2)
        F1 = BETA * W1 + ALPHA * BETA * W2
        F2 = BETA * BETA * W2
        return F0, F1, F2

    E0, E1, E2 = fold(We)
    P0, P1, P2 = fold(Wp)
    Wcat = np.block([[E0.T, P0.T], [E1.T, P1.T], [E2.T, P2.T]])  # [96, 64]
    brow = np.concatenate([2.0 * np.asarray(be), 2.0 * np.asarray(bp)])[None, :]
    Wm = np.concatenate([Wcat, brow], axis=0).astype(np.float16)  # [97, 64]
    return MT, Wm, np.asarray(a, np.float16)


def _install_ntff_shim():
    """Provide antenv.axon_hooks (missing in this image) so
    run_bass_kernel_spmd(trace=True) can drive NTFF profiling via the
    axon PJRT .so. No-op if anything is unavailable."""
    import contextlib
    import ctypes
    import types

    try:
        import antenv  # noqa: F401

        try:
            from antenv.axon_hooks import get_axon_ntff_profile_hook  # noqa: F401

            return
        except ImportError:
            pass
        lib = ctypes.CDLL("/opt/axon/libaxon_pjrt.so")
        if not hasattr(lib, "axon_start_nrt_profile"):
            return
        lib.axon_start_nrt_profile.argtypes = [
            ctypes.POINTER(ctypes.c_int64),
            ctypes.c_size_t,
        ]
        lib.axon_start_nrt_profile.restype = ctypes.c_int64
        lib.axon_stop_nrt_profile.argtypes = [ctypes.c_char_p]
        lib.axon_stop_nrt_profile.restype = ctypes.c_int64

        @contextlib.contextmanager
        def _hook(output_dir, device_ids):
            import jax

            jax.devices()
            if device_ids:
                ids = (ctypes.c_int64 * len(device_ids))(*device_ids)
                rc = lib.axon_start_nrt_profile(ids, len(device_ids))
            else:
                rc = lib.axon_start_nrt_profile(None, 0)
            if rc != 0:
                raise RuntimeError(f"axon_start_nrt_profile rc={rc}")
            try:
                yield
            finally:
                n = lib.axon_stop_nrt_profile(str(output_dir).encode())
                print(f"ntff profile: {n} file(s) -> {output_dir}", file=sys.stderr)

        holder = {"h": _hook}
        mod = types.ModuleType("antenv.axon_hooks")
        mod.get_axon_ntff_profile_hook = lambda: holder["h"]
        mod.set_axon_ntff_profile_hook = lambda h: holder.__setitem__("h", h)
        sys.modules["antenv.axon_hooks"] = mod
        antenv.axon_hooks = mod
    except Exception as e:  # pragma: no cover
        print(f"ntff shim unavailable: {e}", file=sys.stderr)


_NC_CACHE = {}


def _get_nc():
    if "nc" not in _NC_CACHE:
        nc = build_nc()
        nc.compile()
        _NC_CACHE["nc"] = nc
    return _NC_CACHE["nc"]


def run_spmd(x, a, We, be, Wp, bp, trace=False):
    if trace:
        _install_ntff_shim()
    x16 = np.ascontiguousarray(np.asarray(x, np.float16))
    xnm = np.ascontiguousarray(np.transpose(x16, (0, 2, 1, 3)).reshape(
        NCORES, BPC, N, C, L
    ))
    xcm = x16.reshape(NCORES, BPC, C, N, L)
    MT, Wm, a16 = _host_prep(x, a, We, be, Wp, bp)
    nc = _get_nc()
    in_maps = [
        {
            "xnm": xnm[i],
            "xcm": xcm[i],
            "mt": MT,
            "wm": Wm,
            "am": a16,
        }
        for i in range(NCORES)
    ]
    res = run_bass_kernel_spmd(nc, in_maps, list(range(NCORES)), trace=trace)
    xn_s = np.concatenate([res.results[i]["xn"] for i in range(NCORES)], axis=0)
    an_s = np.concatenate([res.results[i]["an"] for i in range(NCORES)], axis=0)
    # [B, L, C, L'] -> [B, C, L, L'] ; [B, L', C, N] -> [B, C, N, L']
    xn = np.ascontiguousarray(np.transpose(xn_s, (0, 2, 1, 3))).astype(np.float32)
    an = np.ascontiguousarray(np.transpose(an_s, (0, 2, 3, 1))).astype(np.float32)
    return (xn, an), res


def kernel(x, a, We, be, Wp, bp):
    (xn, an), _ = run_spmd(x, a, We, be, Wp, bp, trace=False)
    return (xn, an)



# revision 5
# speedup vs baseline: 1.1853x; 1.1778x over previous
"""DiffPool (nn_DiffPool_4715874091424) Trainium2 Bass kernel, v3.

Math (reference jax, B=32, C=CR=32, N=L=160, GDEP=2, ALPHA=0.05):
  A  = rownorm(a + I), A' = rownorm(a.T + I)
  mixprop folding:  [embed; pool] = Wcat^T @ [x; M1 x; M2 x]  (bias handled on
  host: bp drops by softmax shift-invariance, be adds to x_new post-hoc since
  softmax columns sum to 1), with M1 = A + A', M2 = A^2 + A'^2 folded on host.
  s = softmax_v(pool);  x_new[c] = s[c]^T @ embed[c];
  a_new[c] = (s[c] @ a) @ s[c].

v3 performance design (vs v2 baseline):
  - HAM warmth: the PE clock gate re-throttles to K=4/8 (1.2 GHz) after any
    low-duty window and never recovers mid-kernel.  All phases are structured
    for high PE-array duty: stationary-grouped y12/ttg (one LDWEIGHTS per
    3-chunk group), 32-col col-group strips for xnew/anew (concurrent MMs,
    small LDWs), fp8 hcat stationary for mix (FWL probe), plus warm-up /
    bridge / post-mix filler matmuls to cover DMA waits.
  - s^T via DMA xbar transpose (off the PE): xp stored l-padded to 256/chan;
    two dma_start(transpose=True) calls give sx[l, (c, half, v)]; Dinv scale
    on DVE into ss; ttg reads ss views.  PE transposes eliminated.
  - fp8 on the permute path (y tiles, ys scratch, xcm/hcat, wm) halves DMA
    bytes and SBUF; mix matmuls run fp8 stationary+moving.
  Outputs written f16 in [l, c, l'] layouts (contiguous DMA), host transposes
  to [c, l, l'] and casts f32.
"""

import sys

import numpy as np

if "/opt/trn_rl_repo" not in sys.path:
    sys.path.insert(0, "/opt/trn_rl_repo")

import ml_dtypes

import concourse.bass as bass
import concourse.bacc as bacc
import concourse.mybir as mybir
import concourse.tile as tile
from concourse.bass_utils import run_bass_kernel_spmd

F32 = mybir.dt.float32
F16 = mybir.dt.float16
F8 = mybir.dt.float8e4
AF = mybir.ActivationFunctionType
MUL = mybir.AluOpType.mult
NP8 = ml_dtypes.float8_e4m3fn

B, C, N, L = 32, 32, 160, 160
NCORES = 8
BPC = B // NCORES
ALPHA, BETA = 0.05, 0.95
CL = C * L  # 5120
C1 = C * (L + 1)  # 5152
CP = 256  # padded per-channel l-stride for the xbar transpose
CPL = C * CP  # 8192
K96 = 3 * C  # 96 contraction rows (no bias row)
VT = [(0, 128), (128, 32)]
MTILES = [(0, 128), (128, 128), (256, 64)]
G3 = [(c0, min(3, C - c0)) for c0 in range(0, C, 3)]  # channel groups of 3


def build_nc():
    nc = bacc.Bacc("TRN2", target_bir_lowering=False, debug=False, num_devices=NCORES)
    xnm = nc.declare_dram_parameter("xnm", [BPC, N, C, L], F16, isOutput=False)
    xcm = nc.declare_dram_parameter("xcm", [BPC, C, N, L], F16, isOutput=False)
    mt = nc.declare_dram_parameter("mt", [N, 2 * N], F16, isOutput=False)
    wm = nc.declare_dram_parameter("wm", [K96, 2 * C], F16, isOutput=False)
    am = nc.declare_dram_parameter("am", [N, N], F16, isOutput=False)
    xn_out = nc.declare_dram_parameter("xn", [BPC, L, C, L], F16, isOutput=True)
    an_out = nc.declare_dram_parameter("an", [BPC, L, C, N], F16, isOutput=True)
    # chan-major y scratch (node->chan layout rotation via DRAM roundtrip)
    ys = nc.dram_tensor("ys", [BPC, C, 2 * N, L], F8)

    with tile.TileContext(nc) as tc:
        with (
            tc.tile_pool(name="consts", bufs=1) as pc,
            tc.tile_pool(name="work", bufs=1) as pw,
            tc.tile_pool(name="psum", bufs=1, space="PSUM") as pp,
        ):
            # ---- constants (scalar HWDGE queue; sync queue starts with xnm) ----
            mtt = []
            for kt, (k0, ksz) in enumerate(((0, 128), (128, 32))):
                row = []
                for m0, msz in MTILES:
                    t = pc.tile([ksz, msz], F16, name=f"mtt{kt}_{m0}")
                    nc.scalar.dma_start(t[:], mt[k0 : k0 + ksz, m0 : m0 + msz])
                    row.append(t)
                mtt.append(row)
            amt = []
            for kt, (k0, ksz) in enumerate(VT):
                row = []
                for m0, msz in VT:
                    t = pc.tile([ksz, msz], F16, name=f"amt{kt}_{m0}")
                    nc.scalar.dma_start(t[:], am[k0 : k0 + ksz, m0 : m0 + msz])
                    row.append(t)
                amt.append(row)
            wmc = pc.tile([K96, 2 * C], F16)
            nc.scalar.dma_start(wmc[:], wm[:])

            # ---- persistent work tiles (reused across b; bufs=1) ----
            st = {}
            st["xnm0"] = pw.tile([128, CL], F16, tag="xnm0", name="xnm0")
            st["xnm1"] = pw.tile([32, CL], F16, tag="xnm1", name="xnm1")
            st["hcat"] = pw.tile([K96, N * L], F16, tag="hcat", name="hcat")
            st["y0"] = pw.tile([128, CL], F8, tag="y0", name="y0")
            st["y1"] = pw.tile([128, CL], F8, tag="y1", name="y1")
            st["y2"] = pw.tile([64, CL], F8, tag="y2", name="y2")
            st["eg0"] = pw.tile([128, C1], F16, tag="eg0", name="eg0")
            st["eg1"] = pw.tile([32, C1], F16, tag="eg1", name="eg1")
            st["xp0"] = pw.tile([128, CPL], F16, tag="xp0", name="xp0")
            st["xp1"] = pw.tile([32, CPL], F16, tag="xp1", name="xp1")
            st["sx0"] = pw.tile([128, C * 2 * 128], F16, tag="sx0", name="sx0")
            st["sx1"] = pw.tile([128, C * 2 * 32], F16, tag="sx1", name="sx1")
            st["tt0"] = pw.tile([128, CL], F16, tag="tt0", name="tt0")
            st["tt1"] = pw.tile([32, CL], F16, tag="tt1", name="tt1")
            st["ob0"] = pw.tile([128, CL], F16, tag="ob0", name="ob0")
            st["ob1"] = pw.tile([32, CL], F16, tag="ob1", name="ob1")
            st["dv0"] = pw.tile([128, C], F32, tag="dv0", name="dv0")
            st["dv1"] = pw.tile([32, C], F32, tag="dv1", name="dv1")
            st["dh0"] = pw.tile([128, C], F16, tag="dh0", name="dh0")
            st["dh1"] = pw.tile([32, C], F16, tag="dh1", name="dh1")

            # ones column of eg (col 160 of each channel block); constant
            nc.vector.memset(
                st["eg0"][:].rearrange("p (c q) -> p c q", q=L + 1)[:, :, L : L + 1],
                1.0,
            )
            nc.vector.memset(
                st["eg1"][:].rearrange("p (c q) -> p c q", q=L + 1)[:, :, L : L + 1],
                1.0,
            )
            # zero the xp l-pad (cols 160:256 per channel); exp evicts only
            # write l<160 so this stays zero across b
            nc.vector.memset(
                st["xp0"][:].rearrange("p (c q) -> p c q", q=CP)[:, :, L:CP], 0.0
            )
            nc.vector.memset(
                st["xp1"][:].rearrange("p (c q) -> p c q", q=CP)[:, :, L:CP], 0.0
            )

            # PE warm-up: back-to-back matmuls release the HAM clock gate while
            # the first x tiles stream in
            warm = pc.tile([128, 512], F16, name="warm")
            nc.vector.memset(warm[:], 0.125)

            _xin_nm(nc, st, xnm, 0)
            _xin_cm(nc, st, xcm, 0)
            _fill(nc, pp, warm, 52)

            _y12(nc, pp, st, mtt, ys, 0)
            _xin_nm(nc, st, xnm, 1)
            _fill(nc, pp, warm, 56)  # bridge the DRAM-permute(0) wait
            for b in range(BPC):
                if b > 0:
                    _p2(nc, pp, st, amt, xn_out, an_out, b - 1)
                _mix(nc, pp, st, wmc, b)
                _xbar(nc, st)
                # xcm(b+1) AFTER mix(b): it overwrites hcat rows 0:32
                if b + 1 < BPC:
                    _xin_cm(nc, st, xcm, b + 1)
                    _y12(nc, pp, st, mtt, ys, b + 1)
                    if b + 2 < BPC:
                        _xin_nm(nc, st, xnm, b + 2)
            _p2(nc, pp, st, amt, xn_out, an_out, BPC - 1)

    return nc


def _psA(pp, shape, dt=F32):
    return pp.tile(shape, dt, tag="psA", name="psA", bufs=3)


def _psB(pp, shape, dt=F32):
    return pp.tile(shape, dt, tag="psB", name="psB", bufs=3)


def _psS(pp, shape, dt=F32):
    return pp.tile(shape, dt, tag="psS", name="psS", bufs=2)


def _fill(nc, pp, warm, n):
    for _ in range(n):
        wps = _psA(pp, [128, 512])
        nc.tensor.matmul(wps[:], warm[:, 0:128], warm[:], start=True, stop=True)


def _xin_nm(nc, st, xnm, b):
    nc.sync.dma_start(
        st["xnm0"][:].rearrange("p (c l) -> p c l", c=C), xnm[b][0:128]
    )
    nc.sync.dma_start(
        st["xnm1"][:].rearrange("p (c l) -> p c l", c=C), xnm[b][128:160]
    )


def _xin_cm(nc, st, xcm, b):
    # x chan-major -> hcat rows 0:32 (contiguous, fp8)
    nc.scalar.dma_start(
        st["hcat"][0:C, :].rearrange("c (v l) -> c v l", v=N), xcm[b]
    )


def _y12_unit(nc, pp, st, mtt, mi, g0, b):
    """One stationary-grouped y12 unit: <=3 sg chunks, kt0 then kt1."""
    m0, msz = MTILES[mi]
    Y = [st["y0"], st["y1"], st["y2"]]
    xs = [st["xnm0"], st["xnm1"]]
    gsz = min(3, 10 - g0)
    pss = [_psA(pp, [128, 512]) for _ in range(gsz)]
    for kt in range(2):
        for gi in range(gsz):
            sg = g0 + gi
            nc.tensor.matmul(
                pss[gi][:msz, :],
                mtt[kt][mi][:],
                xs[kt][:, sg * 512 : (sg + 1) * 512],
                start=(kt == 0),
                stop=(kt == 1),
            )
    for gi in range(gsz):
        sg = g0 + gi
        nc.scalar.copy(Y[mi][:msz, sg * 512 : (sg + 1) * 512], pss[gi][:msz, :])


def _y12_permute(nc, st, ys, b):
    """y (node-major) -> chan-major hcat rows 32:96 via DRAM roundtrip."""
    hc = st["hcat"]
    nc.gpsimd.dma_start(
        ys[b][:, 0:128, :].rearrange("c v l -> v c l"),
        st["y0"][:].rearrange("v (c l) -> v c l", c=C),
    )
    nc.gpsimd.dma_start(
        ys[b][:, 128:160, :].rearrange("c v l -> v c l"),
        st["y1"][0:32, :].rearrange("v (c l) -> v c l", c=C),
    )
    nc.gpsimd.dma_start(
        ys[b][:, 160:256, :].rearrange("c v l -> v c l"),
        st["y1"][32:128, :].rearrange("v (c l) -> v c l", c=C),
    )
    nc.gpsimd.dma_start(
        ys[b][:, 256:320, :].rearrange("c v l -> v c l"),
        st["y2"][:].rearrange("v (c l) -> v c l", c=C),
    )
    nc.gpsimd.dma_start(
        hc[C : 2 * C, :].rearrange("c (v l) -> c v l", v=N),
        ys[b][:, 0:N, :],
    )
    nc.gpsimd.dma_start(
        hc[2 * C : 3 * C, :].rearrange("c (v l) -> c v l", v=N),
        ys[b][:, N : 2 * N, :],
    )


def _y12(nc, pp, st, mtt, ys, b):
    for mi in range(3):
        for g0 in range(0, 10, 3):
            _y12_unit(nc, pp, st, mtt, mi, g0, b)
    _y12_permute(nc, st, ys, b)


def _mix_lg(nc, pp, st, wmc, vti, lg):
    """One mix unit: 8 l values of one v-tile into a psum bank + evicts."""
    hv = st["hcat"][:].rearrange("p (v l) -> p l v", l=L)
    egs = [st["eg0"], st["eg1"]]
    xps = [st["xp0"], st["xp1"]]
    v0, vsz = VT[vti]
    ps = _psA(pp, [128, 512]) if vti == 0 else _psB(pp, [32, 512])
    for li in range(8):
        l = lg * 8 + li
        nc.tensor.matmul(
            ps[:vsz, li * 64 : (li + 1) * 64],
            hv[:, l, v0 : v0 + vsz],
            wmc[:],
            start=True,
            stop=True,
        )
    pv = ps[:vsz, :].rearrange("p (l c) -> p c l", c=64)
    nc.vector.tensor_copy(
        egs[vti][:].rearrange("p (c q) -> p c q", q=L + 1)[:, :, lg * 8 : lg * 8 + 8],
        pv[:, 0:C, :],
    )
    nc.scalar.activation(
        xps[vti][:].rearrange("p (c q) -> p c q", q=CP)[:, :, lg * 8 : lg * 8 + 8],
        pv[:, C : 2 * C, :],
        AF.Exp,
    )


def _mix(nc, pp, st, wmc, b):
    for vti in range(2):
        for lg in range(20):
            _mix_lg(nc, pp, st, wmc, vti, lg)


def _burst(nc, pp, st, warm, n):
    """Re-warm burst anchored on mix tail evicts (reads xp l-tail regions)."""
    for k in range(n):
        wps = _psA(pp, [128, 512])
        src = st["xp0"] if k % 2 == 0 else st["xp1"]
        c = C - 1 - (k % 4) // 2
        nc.tensor.matmul(
            wps[0:32, :],
            src[0:32, c * CP + 128 : c * CP + 160],
            warm[0:32, :],
            start=True,
            stop=True,
        )


def _mix(nc, pp, st, wmc, b):
    """Channel mix, node-major output: per (l, v-tile) stationary=hcat cols (fp8)."""
    hc = st["hcat"]
    hv = hc[:].rearrange("p (v l) -> p l v", l=L)  # [96, l, v] strided view, fp8
    egs = [st["eg0"], st["eg1"]]
    xps = [st["xp0"], st["xp1"]]
    for vti, (v0, vsz) in enumerate(VT):
        for lg in range(20):  # l-groups of 8 -> one psum bank
            if vti == 0:
                ps = _psA(pp, [128, 512])
            else:
                ps = _psS(pp, [32, 512])
            for li in range(8):
                l = lg * 8 + li
                nc.tensor.matmul(
                    ps[:vsz, li * 64 : (li + 1) * 64],
                    hv[:, l, v0 : v0 + vsz],
                    wmc[:],
                    start=True,
                    stop=True,
                )
            pv = ps[:vsz, :].rearrange("p (l c) -> p c l", c=64)
            # embed rows -> eg (cast, DVE), pool rows -> xp (exp, ACT; 256-padded)
            nc.vector.tensor_copy(
                egs[vti][:]
                .rearrange("p (c q) -> p c q", q=L + 1)[:, :, lg * 8 : lg * 8 + 8],
                pv[:, 0:C, :],
            )
            nc.scalar.activation(
                xps[vti][:]
                .rearrange("p (c q) -> p c q", q=CP)[:, :, lg * 8 : lg * 8 + 8],
                pv[:, C : 2 * C, :],
                AF.Exp,
            )


def _xbar(nc, st):
    """expP^T via DMA xbar transpose: sx[a, b, v] = xp[v, 128*b + a],
    i.e. sx[l%128, (c, l//128), v] given the 256-padded xp layout."""
    nc.scalar.dma_start(
        st["sx0"][:].rearrange("p (b q) -> p b q", q=128),
        st["xp0"][:],
        transpose=True,
    )
    nc.scalar.dma_start(
        st["sx1"][:].rearrange("p (b q) -> p b q", q=32),
        st["xp1"][:],
        transpose=True,
    )


def _sxscale(nc, st, cA, cB, part):
    """sx *= Dinv[l(part), c] in place for channels [cA, cB), one piece.
    part 0/1: sx0 halves k=0/1; part 2: sx1 both halves."""
    nbc = cB - cA
    if part < 2:
        sv = st["sx0"][:].rearrange("p (c k v) -> p c k v", k=2, v=128)
        if part == 0:
            in0 = st["dh0"][:, cA:cB].unsqueeze(2).broadcast_to([128, nbc, 128])
            nc.vector.tensor_tensor(sv[:, cA:cB, 0, :], sv[:, cA:cB, 0, :], in0, MUL)
        else:
            in1 = st["dh1"][:, cA:cB].unsqueeze(2).broadcast_to([32, nbc, 128])
            nc.vector.tensor_tensor(
                sv[0:32, cA:cB, 1, :], sv[0:32, cA:cB, 1, :], in1, MUL
            )
    else:
        sv = st["sx1"][:].rearrange("p (c k v) -> p c k v", k=2, v=32)
        in0 = st["dh0"][:, cA:cB].unsqueeze(2).broadcast_to([128, nbc, 32])
        nc.vector.tensor_tensor(sv[:, cA:cB, 0, :], sv[:, cA:cB, 0, :], in0, MUL)
        in1 = st["dh1"][:, cA:cB].unsqueeze(2).broadcast_to([32, nbc, 32])
        nc.vector.tensor_tensor(
            sv[0:32, cA:cB, 1, :], sv[0:32, cA:cB, 1, :], in1, MUL
        )


def _xnew(nc, pp, st, xn_out, b):
    """x_new raw = expP^T @ [eg|1] via 32-col strips; Dinv recip + row-scale."""
    egs = [st["eg0"], st["eg1"]]
    xpt = [st["xp0"], st["xp1"]]
    obs = [st["ob0"], st["ob1"]]
    dvs = [st["dv0"], st["dv1"]]
    for g, (c0, gc) in enumerate(G3):
        pl = [_psB(pp, [128, 512]), _psS(pp, [32, 512])]
        for ci in range(gc):
            c = c0 + ci
            for kt in range(2):
                for s in range(4):
                    nc.tensor.matmul(
                        pl[0][32 * s : 32 * s + 32, ci * 161 : ci * 161 + 161],
                        xpt[kt][:, c * CP + 32 * s : c * CP + 32 * s + 32],
                        egs[kt][:, c * (L + 1) : (c + 1) * (L + 1)],
                        start=(kt == 0),
                        stop=(kt == 1),
                        tile_position=(0, 32 * s),
                    )
            for kt in range(2):
                nc.tensor.matmul(
                    pl[1][0:32, ci * 161 : ci * 161 + 161],
                    xpt[kt][:, c * CP + 128 : c * CP + 160],
                    egs[kt][:, c * (L + 1) : (c + 1) * (L + 1)],
                    start=(kt == 0),
                    stop=(kt == 1),
                )
        for lt, (l0, lsz) in enumerate(VT):
            pq = pl[lt][:lsz, : gc * 161].rearrange("p (c q) -> p c q", q=161)
            nc.vector.reciprocal(
                dvs[lt][:lsz, c0 : c0 + gc],
                pq[:, :, 160:161].rearrange("p c q -> p (c q)"),
            )
            in1 = (
                dvs[lt][:lsz, c0 : c0 + gc].unsqueeze(2).broadcast_to([lsz, gc, L])
            )
            nc.vector.tensor_tensor(
                obs[lt][:lsz, c0 * L : (c0 + gc) * L].rearrange(
                    "p (c q) -> p c q", q=L
                ),
                pq[:, :, 0:L],
                in1,
                MUL,
            )
        if g == 5:  # dv ready for c 0:18 -> dh copies for first half
            for lt, (l0, lsz) in enumerate(VT):
                nc.vector.tensor_copy(
                    [st["dh0"], st["dh1"]][lt][:lsz, 0:16],
                    dvs[lt][:lsz, 0:16],
                )
        elif 6 <= g <= 8:  # spread the first-half scales over groups 6-8
            _sxscale(nc, st, 0, 16, g - 6)
    for lt, (l0, lsz) in enumerate(VT):
        nc.vector.tensor_copy(
            [st["dh0"], st["dh1"]][lt][:lsz, 16:C], dvs[lt][:lsz, 16:C]
        )
    for part in range(3):
        _sxscale(nc, st, 16, C, part)

    # ---- xn out (obs holds x_new now) ----
    nc.sync.dma_start(
        xn_out[b][0:128], obs[0][:].rearrange("p (c l) -> p c l", l=L)
    )
    nc.sync.dma_start(
        xn_out[b][128:160], obs[1][:].rearrange("p (c l) -> p c l", l=L)
    )


def _ttg(nc, pp, st, amt):
    """ttg = a^T s^T, stationary-grouped over 3-chunk groups; moving from ss."""
    s0 = st["sx0"][:].rearrange("p (c k v) -> p c k v", k=2, v=128)
    s1 = st["sx1"][:].rearrange("p (c k v) -> p c k v", k=2, v=32)
    tts = [st["tt0"], st["tt1"]]
    chunks = [("A", i) for i in range(8)] + [("B", i) for i in range(2)]

    def rhs(kt, typ, i):
        if typ == "A":
            if kt == 0:
                return s0[:, 4 * i : 4 * i + 4, 0, :]
            return s0[0:32, 4 * i : 4 * i + 4, 1, :]
        if kt == 0:
            return s1[:, 16 * i : 16 * i + 16, 0, :]
        return s1[0:32, 16 * i : 16 * i + 16, 1, :]

    for mi, (m0, msz) in enumerate(VT):
        ttv = tts[mi][:].rearrange("p (c w) -> p c w", w=L)
        for g0 in range(0, 10, 3):
            grp = chunks[g0 : g0 + 3]
            pss = [_psB(pp, [128, 512]) for _ in grp]
            for kt in range(2):
                for gi, (typ, i) in enumerate(grp):
                    nc.tensor.matmul(
                        pss[gi][:msz, :],
                        amt[kt][mi][:],
                        rhs(kt, typ, i),
                        start=(kt == 0),
                        stop=(kt == 1),
                    )
            for gi, (typ, i) in enumerate(grp):
                if typ == "A":
                    nc.scalar.copy(
                        ttv[:, 4 * i : 4 * i + 4, 0:128],
                        pss[gi][:msz, :].rearrange("p (c v) -> p c v", v=128),
                    )
                else:
                    nc.scalar.copy(
                        ttv[:, 16 * i : 16 * i + 16, 128:160],
                        pss[gi][:msz, :].rearrange("p (c v) -> p c v", v=32),
                    )


def _anew(nc, pp, st, an_out, b):
    """a_new^T raw = expP^T @ ttg via 32-col strips; Dinv row-scale at evict."""
    xpt = [st["xp0"], st["xp1"]]
    tts = [st["tt0"], st["tt1"]]
    obs = [st["ob0"], st["ob1"]]
    dvs = [st["dv0"], st["dv1"]]
    for c0, gc in G3:
        pl = [_psB(pp, [128, 512]), _psS(pp, [32, 512])]
        for ci in range(gc):
            c = c0 + ci
            for jt in range(2):
                for s in range(4):
                    nc.tensor.matmul(
                        pl[0][32 * s : 32 * s + 32, ci * N : (ci + 1) * N],
                        xpt[jt][:, c * CP + 32 * s : c * CP + 32 * s + 32],
                        tts[jt][:, c * N : (c + 1) * N],
                        start=(jt == 0),
                        stop=(jt == 1),
                        tile_position=(0, 32 * s),
                    )
            for jt in range(2):
                nc.tensor.matmul(
                    pl[1][0:32, ci * N : (ci + 1) * N],
                    xpt[jt][:, c * CP + 128 : c * CP + 160],
                    tts[jt][:, c * N : (c + 1) * N],
                    start=(jt == 0),
                    stop=(jt == 1),
                )
        for lt, (l0, lsz) in enumerate(VT):
            in1 = (
                dvs[lt][:lsz, c0 : c0 + gc].unsqueeze(2).broadcast_to([lsz, gc, N])
            )
            nc.vector.tensor_tensor(
                obs[lt][:lsz, c0 * N : (c0 + gc) * N].rearrange(
                    "p (c q) -> p c q", q=N
                ),
                pl[lt][:lsz, : gc * N].rearrange("p (c q) -> p c q", q=N),
                in1,
                MUL,
            )

    # ---- an out ----
    nc.sync.dma_start(
        an_out[b][0:128], obs[0][:].rearrange("p (c l) -> p c l", l=N)
    )
    nc.sync.dma_start(
        an_out[b][128:160], obs[1][:].rearrange("p (c l) -> p c l", l=N)
    )


def _p2(nc, pp, st, amt, xn_out, an_out, b):
    _xnew(nc, pp, st, xn_out, b)
    _ttg(nc, pp, st, amt)
    _anew(nc, pp, st, an_out, b)


def _host_prep(x, a, We, be, Wp, bp):
    a = np.asarray(a, np.float64)
    I = np.eye(N, dtype=np.float64)
    A1 = (a + I) / (a + I).sum(1, keepdims=True)
    A2 = (a.T + I) / (a.T + I).sum(1, keepdims=True)
    M1 = A1 + A2
    M2 = A1 @ A1 + A2 @ A2
    MT = np.concatenate([M1.T, M2.T], axis=1).astype(np.float16)  # [N, 2N]

    def fold(W):
        W = np.asarray(W, np.float64)
        W0, W1, W2 = W[:, :C], W[:, C : 2 * C], W[:, 2 * C :]
        F0 = 2.0 * (W0 + ALPHA * W1 + ALPHA * W2)
        F1 = BETA * W1 + ALPHA * BETA * W2
        F2 = BETA * BETA * W2
        return F0, F1, F2

    E0, E1, E2 = fold(We)
    P0, P1, P2 = fold(Wp)
    Wcat = np.block([[E0.T, P0.T], [E1.T, P1.T], [E2.T, P2.T]])  # [96, 64]
    return MT, Wcat.astype(np.float16), np.asarray(a, np.float16)


def _install_ntff_shim():
    """Provide antenv.axon_hooks (missing in this image) so
    run_bass_kernel_spmd(trace=True) can drive NTFF profiling via the
    axon PJRT .so. No-op if anything is unavailable."""
    import contextlib
    import ctypes
    import types

    try:
        import antenv  # noqa: F401

        try:
            from antenv.axon_hooks import get_axon_ntff_profile_hook  # noqa: F401

            return
        except ImportError:
            pass
        lib = ctypes.CDLL("/opt/axon/libaxon_pjrt.so")
        if not hasattr(lib, "axon_start_nrt_profile"):
            return
        lib.axon_start_nrt_profile.argtypes = [
            ctypes.POINTER(ctypes.c_int64),
            ctypes.c_size_t,
        ]
        lib.axon_start_nrt_profile.restype = ctypes.c_int64
        lib.axon_stop_nrt_profile.argtypes = [ctypes.c_char_p]
        lib.axon_stop_nrt_profile.restype = ctypes.c_int64

        @contextlib.contextmanager
        def _hook(output_dir, device_ids):
            import jax

            jax.devices()
            if device_ids:
                ids = (ctypes.c_int64 * len(device_ids))(*device_ids)
                rc = lib.axon_start_nrt_profile(ids, len(device_ids))
            else:
                rc = lib.axon_start_nrt_profile(None, 0)
            if rc != 0:
                raise RuntimeError(f"axon_start_nrt_profile rc={rc}")
            try:
                yield
            finally:
                n = lib.axon_stop_nrt_profile(str(output_dir).encode())
                print(f"ntff profile: {n} file(s) -> {output_dir}", file=sys.stderr)

        holder = {"h": _hook}
        mod = types.ModuleType("antenv.axon_hooks")
        mod.get_axon_ntff_profile_hook = lambda: holder["h"]
        mod.set_axon_ntff_profile_hook = lambda h: holder.__setitem__("h", h)
        sys.modules["antenv.axon_hooks"] = mod
        antenv.axon_hooks = mod
    except Exception as e:  # pragma: no cover
        print(f"ntff shim unavailable: {e}", file=sys.stderr)


_NC_CACHE = {}


def _get_nc():
    if "nc" not in _NC_CACHE:
        nc = build_nc()
        nc.compile()
        _NC_CACHE["nc"] = nc
    return _NC_CACHE["nc"]


def run_spmd(x, a, We, be, Wp, bp, trace=False):
    if trace:
        _install_ntff_shim()
    x16 = np.ascontiguousarray(np.asarray(x, np.float16))
    xnm = np.ascontiguousarray(
        np.transpose(x16, (0, 2, 1, 3)).reshape(NCORES, BPC, N, C, L)
    )
    xcm = x16.reshape(NCORES, BPC, C, N, L)
    MT, Wm8, a16 = _host_prep(x, a, We, be, Wp, bp)
    nc = _get_nc()
    in_maps = [
        {
            "xnm": xnm[i],
            "xcm": xcm[i],
            "mt": MT,
            "wm": Wm8,
            "am": a16,
        }
        for i in range(NCORES)
    ]
    res = run_bass_kernel_spmd(nc, in_maps, list(range(NCORES)), trace=trace)
    xn_s = np.concatenate([res.results[i]["xn"] for i in range(NCORES)], axis=0)
    an_s = np.concatenate([res.results[i]["an"] for i in range(NCORES)], axis=0)
    # [B, L, C, L'] -> [B, C, L, L'] ; [B, L', C, N] -> [B, C, N, L']
    xn = np.ascontiguousarray(np.transpose(xn_s, (0, 2, 1, 3))).astype(np.float32)
    xn += np.asarray(be, np.float32)[None, :, None, None]
    an = np.ascontiguousarray(np.transpose(an_s, (0, 2, 3, 1))).astype(np.float32)
    return (xn, an), res


def kernel(x, a, We, be, Wp, bp):
    (xn, an), _ = run_spmd(x, a, We, be, Wp, bp, trace=False)
    return (xn, an)
